# revision 55
# baseline (speedup 1.0000x reference)
"""Distributed ring-attention kernel for Trainium2 (8 NeuronCores, Bass/Tile).

Strategy (seq-parallel attention, full softmax without max-subtraction):
  - Host: transpose/cast inputs to bf16; shard x.T column-wise (seq) across 8 cores.
  - Per core: project Q/K/V for its 512-seq shard; AllGather K^T and V
    across cores (split into per-head-pair collectives so attention on the
    first pairs overlaps the remaining transfers); compute full attention
    for its Q shard over the whole 4096-length K/V; out-projection; write
    its y shard.
  - Scores are computed transposed (S^T = K @ Q^T, kpos on partitions) so the
    exp'd probabilities feed the P@V matmul directly as the stationary-side
    contraction. Softmax denominator comes for free from a ones-column
    appended to V. Softmax skips max-subtraction: scores are O(1) here
    (exp is numerically safe), which matches softmax exactly in exact math.
"""

import numpy as np
import ml_dtypes

HID = 1024
HEADS = 16
HD = 64
S = 4096
NCORES = 8
SQ = S // NCORES          # 512 q rows per core
PAIRS = HEADS // 2        # 8 head pairs (128 rows of qkvT per pair)
KTILES = S // 128         # 32 kpos tiles per head
VAUG = HD + 1             # 65: V plus ones column
SCALE = 1.0 / np.sqrt(HD)

_cache = {}


def _build():
    import concourse.bass as bass
    import concourse.mybir as mybir
    import concourse.tile as tile
    from concourse import bacc

    dt = mybir.dt
    nc = bacc.Bacc("TRN2", target_bir_lowering=False, debug=False,
                   num_devices=NCORES)

    xT = nc.dram_tensor("xT", [HID, SQ], dt.bfloat16, kind="ExternalInput").ap()
    wqkvT = nc.dram_tensor("wqkvT", [HID, 3 * HID], dt.bfloat16,
                           kind="ExternalInput").ap()
    woutT = nc.dram_tensor("woutT", [HID, HID], dt.bfloat16,
                           kind="ExternalInput").ap()
    y = nc.dram_tensor("y", [SQ, HID], dt.float32, kind="ExternalOutput").ap()

    with tile.TileContext(nc) as tc:
        _body(nc, tc, bass, mybir, xT, wqkvT, woutT, y)

    nc.compile()
    return nc


def _body(nc, tc, bass, mybir, xT, wqkvT, woutT, y):
    dt = mybir.dt
    f32, bf16, f8 = dt.float32, dt.bfloat16, dt.float8e4
    RG = [list(range(NCORES))]

    with (
        tc.tile_pool(name="dram", bufs=1, space="DRAM") as dram,
        tc.tile_pool(name="resident", bufs=1) as res,
        tc.tile_pool(name="stream", bufs=1) as st,
    ):
        # ---- DRAM bounce buffers for collectives, one set per UNIT of
        # head pairs; first units are single pairs so attention can start
        # as soon as possible ----
        UNITS = [[0, 1], [2, 3], [4, 5], [6, 7]]
        unit_of = {}
        for u, prs in enumerate(UNITS):
            for i, p in enumerate(prs):
                unit_of[p] = (u, i)
        ktb, vb, ktg, vg = [], [], [], []
        for u, prs in enumerate(UNITS):
            n = len(prs)
            ktb.append(dram.tile([n * 128, SQ], bf16, name=f"ktb{u}"))
            vb.append(dram.tile([SQ, n * 2 * HD], bf16, name=f"vb{u}"))
            ktg.append(dram.tile([NCORES * n * 128, SQ], bf16,
                                 addr_space="Shared", name=f"ktg{u}"))
            vg.append(dram.tile([S, n * 2 * HD], bf16, addr_space="Shared",
                                name=f"vg{u}"))

        # ---- load xT (hidden x local-seq), 8 resident tiles ----
        xt = []
        for k in range(8):
            t = res.tile([128, SQ], bf16, tag=f"xt{k}", name=f"xt{k}")
            nc.sync.dma_start(t[:], xT[k * 128:(k + 1) * 128, :])
            xt.append(t)

        # wqkvT strip views for batched weight loads
        wq4 = wqkvT.rearrange("(k p) (m c) -> p m k c", p=128, c=128)
        wv4 = wqkvT.rearrange("(k p) (m c) -> p m k c", p=128, c=512)

        def kt_proj(m, psP):
            """K^T rows for pair m (qkvT rows 1024+m*128) -> its unit."""
            u, i = unit_of[m]
            ws = st.tile([128, 8 * 128], bf16, tag="wl", bufs=3)
            nc.sync.dma_start(ws.rearrange("p (k c) -> p k c", c=128),
                              wq4[:, 8 + m, :, :])
            ps = psP.tile([128, SQ], f32, tag="proj", bufs=2)
            for k in range(8):
                nc.tensor.matmul(ps[:], ws[:, k * 128:(k + 1) * 128],
                                 xt[k][:], start=(k == 0), stop=(k == 7))
            sb = st.tile([128, SQ], bf16, tag="kt_stage", bufs=3)
            nc.vector.tensor_copy(sb[:], ps[:])
            nc.sync.dma_start(ktb[u][i * 128:(i + 1) * 128, :], sb[:])
            if i == len(UNITS[u]) - 1:
                nc.gpsimd.collective_compute(
                    "AllGather", mybir.AluOpType.bypass, replica_groups=RG,
                    ins=[ktb[u].opt()], outs=[ktg[u].opt()])

        wv2 = wqkvT.rearrange("(k p) (m c) -> p m k c", p=128, c=128)

        def v_proj(u, psP):
            """V rows (natural [s,d]) for unit u's pairs -> vb[u] + AG.

            One psum group per s-tile, N = 128 * n_pairs (<=512)."""
            prs = UNITS[u]
            n = len(prs)
            wvs = st.tile([128, 8 * n * 128], bf16, tag="wvs", bufs=2)
            wvs3 = wvs.rearrange("p (k c) -> p k c", c=n * 128)
            nc.sync.dma_start(
                wvs3.rearrange("p k (pr c) -> p k pr c", c=128),
                wv2[:, 16 + prs[0]:16 + prs[0] + n, :, :].rearrange(
                    "p pr k c -> p k pr c"))
            for sti in range(4):
                ps = psP.tile([128, n * 128], f32, tag="proj", bufs=2)
                for k in range(8):
                    nc.tensor.matmul(
                        ps[:], xt[k][:, sti * 128:(sti + 1) * 128],
                        wvs[:, k * n * 128:(k + 1) * n * 128],
                        start=(k == 0), stop=(k == 7))
                sb = st.tile([128, n * 128], bf16, tag="kv_stage", bufs=3)
                nc.vector.tensor_copy(sb[:], ps[:])
                nc.sync.dma_start(vb[u][sti * 128:(sti + 1) * 128, :], sb[:])
            nc.gpsimd.collective_compute(
                "AllGather", mybir.AluOpType.bypass, replica_groups=RG,
                ins=[vb[u].opt()], outs=[vg[u].opt()])

        qt = [None] * PAIRS

        def q_proj(m, psP):
            ws = st.tile([128, 8 * 128], bf16, tag="wl", bufs=3)
            nc.sync.dma_start(ws.rearrange("p (k c) -> p k c", c=128),
                              wq4[:, m, :, :])
            ps = psP.tile([128, SQ], f32, tag="proj", bufs=2)
            for k in range(8):
                nc.tensor.matmul(ps[:], ws[:, k * 128:(k + 1) * 128],
                                 xt[k][:], start=(k == 0), stop=(k == 7))
            t = res.tile([128, SQ], bf16, tag=f"qt{m}", name=f"qt{m}")
            nc.vector.tensor_copy(t[:], ps[:])
            qt[m] = t

        with tc.tile_pool(name="psP", bufs=1, space="PSUM") as psP:
            # emit each unit's kT then V so the collectives fire in exactly
            # the order attention consumes them
            kt_proj(0, psP)
            kt_proj(1, psP)
            v_proj(0, psP)
            kt_proj(2, psP)
            kt_proj(3, psP)
            v_proj(1, psP)
            q_proj(0, psP)
            kt_proj(4, psP)
            kt_proj(5, psP)
            v_proj(2, psP)
            kt_proj(6, psP)
            kt_proj(7, psP)
            v_proj(3, psP)
            for m in range(1, PAIRS):
                q_proj(m, psP)

        # ---- attention (head pairs row-packed on the PE array) ----
        # pair slabs: rows 0..63 even head, 64..127 odd head
        attn = []
        for p in range(PAIRS):
            t = res.tile([128, SQ], bf16, tag=f"attn{p}", name=f"attn{p}")
            attn.append(t)

        # out-projection weights (pair-stacked rows: odd heads at
        # partitions 64..127) and SBUF accumulators for incremental y
        wo5 = woutT.rearrange("(pp r) (o c) -> r o pp c", r=128, c=512)
        wo = []
        for och in range(2):
            w = res.tile([128, PAIRS * 512], bf16, tag=f"wo{och}",
                         name=f"wo{och}")
            nc.sync.dma_start(
                w.rearrange("r (pp c) -> r pp c", c=512), wo5[:, och])
            wo.append(w)
        with tc.tile_pool(name="psA", bufs=1, space="PSUM") as psA:
            RA = 8
            # global group stream across pairs: PV emission lags the
            # score/exp stream by RA groups even across pair boundaries,
            # so the in-order PE queue always holds gather-independent
            # scores ahead of possibly-blocked PV matmuls
            stream = []
            for p in range(PAIRS):
                slots = [(t, e) for t in range(KTILES) for e in range(2)]
                for gs in range(0, len(slots), 3):
                    stream.append((p, slots[gs:gs + 3]))
            last_of = {}
            for i, (p, _) in enumerate(stream):
                last_of[p] = i

            ctx = {}
            pts = [None] * len(stream)

            def start_pair(p):
                u, i = unit_of[p]
                n = len(UNITS[u])
                ktg3 = ktg[u].rearrange("(c i r) q -> i r c q", i=n, r=128)
                kth = st.tile([128, S], bf16, tag="kth", bufs=3)
                nc.gpsimd.dma_start(
                    kth.rearrange("r (c q) -> r c q", q=SQ), ktg3[i])
                vg3 = vg[u].rearrange("(t q) (i hh c) -> i hh q t c", q=128,
                                      i=n, c=HD)[i]
                vah = []
                for e in range(2):
                    va = st.tile([128, KTILES * VAUG], bf16, tag="vah",
                                 bufs=6)
                    nc.vector.memset(va[:], 1.0)
                    nc.gpsimd.dma_start(
                        va.rearrange("q (t c) -> q t c", c=VAUG)[:, :, 0:HD],
                        vg3[e])
                    vah.append(va)
                pv = [psA.tile([128, 512], f32, tag="pv", bufs=2,
                               name=f"pv{p}_{e}") for e in range(2)]
                ctx[p] = (kth, vah, pv)

            def emit_scores(p, group, si):
                kth = ctx[p][0]
                gw = 512 * len(group)
                sc = psA.tile([128, 1536], f32, tag="sc", bufs=2)
                for idx, (t, e) in enumerate(group):
                    nc.tensor.matmul(
                        sc[:, idx * 512:(idx + 1) * 512],
                        kth[e * 64:(e + 1) * 64, t * 128:(t + 1) * 128],
                        qt[p][e * 64:(e + 1) * 64, :],
                        start=True, stop=True,
                        tile_position=(e * 64, 0))
                pt = st.tile([128, 1536], bf16, tag="pt", bufs=RA + 2)
                nc.scalar.activation(pt[:, 0:gw], sc[:, 0:gw],
                                     mybir.ActivationFunctionType.Exp,
                                     scale=float(SCALE))
                pts[si] = pt

            def emit_pv(p, group, pt):
                vah, pv = ctx[p][1], ctx[p][2]
                for idx, (t, e) in enumerate(group):
                    nc.tensor.matmul(
                        pv[e][0:VAUG, :],
                        vah[e][:, t * VAUG:(t + 1) * VAUG],
                        pt[:, idx * 512:(idx + 1) * 512],
                        start=(t == 0), stop=(t == KTILES - 1))

            def finish_pair(p):
                pv = ctx[p][2]
                for e in range(2):
                    pvs = st.tile([VAUG, 512], f32, tag="pvs", bufs=4)
                    nc.vector.tensor_copy(pvs[:], pv[e][0:VAUG, :])
                    l0 = st.tile([1, 512], f32, tag="l0", bufs=2)
                    nc.sync.dma_start(l0[:], pvs[64:65, :])
                    lb = st.tile([64, 512], f32, tag="lb", bufs=2)
                    nc.gpsimd.partition_broadcast(lb[:], l0[:])
                    rb = st.tile([64, 512], f32, tag="rb", bufs=2)
                    nc.vector.reciprocal_approx_fast(rb[:], lb[:])
                    if e == 0:
                        nc.vector.tensor_mul(attn[p][0:64, :],
                                             pvs[0:64, :], rb[:])
                    else:
                        ao = st.tile([64, SQ], bf16, tag="ao", bufs=2)
                        nc.vector.tensor_mul(ao[:], pvs[0:64, :], rb[:])
                        nc.sync.dma_start(attn[p][64:128, :], ao[:])

            def consume(j):
                pj, gj = stream[j]
                emit_pv(pj, gj, pts[j])
                if j == last_of[pj]:
                    finish_pair(pj)

            for i, (p, group) in enumerate(stream):
                if p not in ctx:
                    start_pair(p)
                emit_scores(p, group, i)
                if i >= RA:
                    consume(i - RA)
            for j in range(len(stream) - RA, len(stream)):
                consume(j)

        # ---- out projection: y[s, o] = sum_h attn_h^T.T @ woutT[h rows],
        # row-packed pairs -> two accumulators (even/odd rows), then add ----
        with tc.tile_pool(name="psY", bufs=1, space="PSUM") as psY:
            for sti in range(4):
                for och in range(2):
                    psa = psY.tile([128, 512], f32, tag="ya", bufs=4)
                    psb = psY.tile([128, 512], f32, tag="yb", bufs=4)
                    for p in range(PAIRS):
                        nc.tensor.matmul(
                            psa[:], attn[p][0:64, sti * 128:(sti + 1) * 128],
                            wo[och][0:64, p * 512:(p + 1) * 512],
                            start=(p == 0), stop=(p == PAIRS - 1),
                            tile_position=(0, 0))
                        nc.tensor.matmul(
                            psb[:], attn[p][64:128, sti * 128:(sti + 1) * 128],
                            wo[och][64:128, p * 512:(p + 1) * 512],
                            start=(p == 0), stop=(p == PAIRS - 1),
                            tile_position=(64, 0))
                    ya = st.tile([128, 512], f32, tag="ya_sb", bufs=3)
                    nc.vector.tensor_copy(ya[:], psa[:])
                    ysb = st.tile([128, 512], f32, tag="ysb", bufs=3)
                    nc.vector.tensor_add(ysb[:], psb[:], ya[:])
                    nc.sync.dma_start(
                        y[sti * 128:(sti + 1) * 128,
                          och * 512:(och + 1) * 512], ysb[:])


def _get_nc():
    if "nc" not in _cache:
        _cache["nc"] = _build()
    return _cache["nc"]


def kernel(x, W_qkv, W_out, _trace=False):
    from concourse.bass_utils import run_bass_kernel_spmd

    nc = _get_nc()
    bf16 = ml_dtypes.bfloat16

    x = np.asarray(x)
    xTf = np.ascontiguousarray(x.reshape(S, HID).T).astype(bf16)   # [HID, S]
    wqkvT = np.ascontiguousarray(np.asarray(W_qkv).T).astype(bf16)
    woutT = np.ascontiguousarray(np.asarray(W_out).T).astype(bf16)

    in_maps = []
    for c in range(NCORES):
        in_maps.append({
            "xT": np.ascontiguousarray(xTf[:, c * SQ:(c + 1) * SQ]),
            "wqkvT": wqkvT,
            "woutT": woutT,
        })
    res = run_bass_kernel_spmd(nc, in_maps, core_ids=list(range(NCORES)),
                               trace=_trace)
    out = np.concatenate([res.results[c]["y"] for c in range(NCORES)],
                         axis=0)
    out = out.reshape(1, S, HID).astype(np.float32)
    if _trace:
        kernel.last_results = res
    return out


# revision 56
# speedup vs baseline: 1.2848x; 1.2848x over previous
"""Distributed ring-attention kernel for Trainium2 (8 NeuronCores, Bass/Tile).

Strategy (seq-parallel attention, full softmax without max-subtraction):
  - Host: transpose/cast inputs to bf16; shard x.T column-wise (seq) across 8 cores.
  - Per core: project Q/K/V for its 512-seq shard; AllGather K^T and V
    across cores (split into per-head-pair collectives so attention on the
    first pairs overlaps the remaining transfers); compute full attention
    for its Q shard over the whole 4096-length K/V; out-projection; write
    its y shard.
  - Scores are computed transposed (S^T = K @ Q^T, kpos on partitions) so the
    exp'd probabilities feed the P@V matmul directly as the stationary-side
    contraction. Softmax denominator comes for free from a ones-column
    appended to V. Softmax skips max-subtraction: scores are O(1) here
    (exp is numerically safe), which matches softmax exactly in exact math.
"""

import numpy as np
import ml_dtypes

HID = 1024
HEADS = 16
HD = 64
S = 4096
NCORES = 8
SQ = S // NCORES          # 512 q rows per core
PAIRS = HEADS // 2        # 8 head pairs (128 rows of qkvT per pair)
KTILES = S // 128         # 32 kpos tiles per head
VAUG = HD + 1             # 65: V plus ones column
SCALE = 1.0 / np.sqrt(HD)

_cache = {}


def _build():
    import concourse.bass as bass
    import concourse.mybir as mybir
    import concourse.tile as tile
    from concourse import bacc

    dt = mybir.dt
    nc = bacc.Bacc("TRN2", target_bir_lowering=False, debug=False,
                   num_devices=NCORES)

    xT = nc.dram_tensor("xT", [HID, SQ], dt.bfloat16, kind="ExternalInput").ap()
    wqkvT = nc.dram_tensor("wqkvT", [HID, 3 * HID], dt.bfloat16,
                           kind="ExternalInput").ap()
    woutT = nc.dram_tensor("woutT", [HID, HID], dt.bfloat16,
                           kind="ExternalInput").ap()
    y = nc.dram_tensor("y", [SQ, HID], dt.float32, kind="ExternalOutput").ap()

    with tile.TileContext(nc) as tc:
        _body(nc, tc, bass, mybir, xT, wqkvT, woutT, y)

    nc.compile()
    return nc


def _body(nc, tc, bass, mybir, xT, wqkvT, woutT, y):
    dt = mybir.dt
    f32, bf16, f8 = dt.float32, dt.bfloat16, dt.float8e4
    RG = [list(range(NCORES))]

    with (
        tc.tile_pool(name="dram", bufs=1, space="DRAM") as dram,
        tc.tile_pool(name="resident", bufs=1) as res,
        tc.tile_pool(name="stream", bufs=1) as st,
    ):
        # ---- DRAM bounce buffers for collectives, one set per UNIT of
        # head pairs; first units are single pairs so attention can start
        # as soon as possible ----
        UNITS = [[0, 1], [2, 3], [4, 5], [6, 7]]
        unit_of = {}
        for u, prs in enumerate(UNITS):
            for i, p in enumerate(prs):
                unit_of[p] = (u, i)
        ktb, vb, ktg, vg = [], [], [], []
        for u, prs in enumerate(UNITS):
            n = len(prs)
            ktb.append(dram.tile([n * 128, SQ], bf16, name=f"ktb{u}"))
            vb.append(dram.tile([SQ, n * 2 * HD], bf16, name=f"vb{u}"))
            ktg.append(dram.tile([NCORES * n * 128, SQ], bf16,
                                 addr_space="Shared", name=f"ktg{u}"))
            vg.append(dram.tile([S, n * 2 * HD], bf16, addr_space="Shared",
                                name=f"vg{u}"))

        # ---- load xT (hidden x local-seq), 8 resident tiles ----
        xt = []
        for k in range(8):
            t = res.tile([128, SQ], bf16, tag=f"xt{k}", name=f"xt{k}")
            nc.sync.dma_start(t[:], xT[k * 128:(k + 1) * 128, :])
            xt.append(t)

        # wqkvT strip views for batched weight loads
        wq4 = wqkvT.rearrange("(k p) (m c) -> p m k c", p=128, c=128)
        wv4 = wqkvT.rearrange("(k p) (m c) -> p m k c", p=128, c=512)

        def kt_proj(m, psP):
            """K^T rows for pair m (qkvT rows 1024+m*128) -> its unit."""
            u, i = unit_of[m]
            ws = st.tile([128, 8 * 128], bf16, tag="wl", bufs=3)
            nc.sync.dma_start(ws.rearrange("p (k c) -> p k c", c=128),
                              wq4[:, 8 + m, :, :])
            ps = psP.tile([128, SQ], f32, tag="proj", bufs=2)
            for k in range(8):
                nc.tensor.matmul(ps[:], ws[:, k * 128:(k + 1) * 128],
                                 xt[k][:], start=(k == 0), stop=(k == 7))
            sb = st.tile([128, SQ], bf16, tag="kt_stage", bufs=3)
            nc.vector.tensor_copy(sb[:], ps[:])
            nc.sync.dma_start(ktb[u][i * 128:(i + 1) * 128, :], sb[:])
            if i == len(UNITS[u]) - 1:
                nc.gpsimd.collective_compute(
                    "AllGather", mybir.AluOpType.bypass, replica_groups=RG,
                    ins=[ktb[u].opt()], outs=[ktg[u].opt()])

        wv2 = wqkvT.rearrange("(k p) (m c) -> p m k c", p=128, c=128)

        def v_proj(u, psP):
            """V rows (natural [s,d]) for unit u's pairs -> vb[u] + AG.

            One psum group per s-tile, N = 128 * n_pairs (<=512)."""
            prs = UNITS[u]
            n = len(prs)
            wvs = st.tile([128, 8 * n * 128], bf16, tag="wvs", bufs=2)
            wvs3 = wvs.rearrange("p (k c) -> p k c", c=n * 128)
            nc.sync.dma_start(
                wvs3.rearrange("p k (pr c) -> p k pr c", c=128),
                wv2[:, 16 + prs[0]:16 + prs[0] + n, :, :].rearrange(
                    "p pr k c -> p k pr c"))
            for sti in range(4):
                ps = psP.tile([128, n * 128], f32, tag="proj", bufs=2)
                for k in range(8):
                    nc.tensor.matmul(
                        ps[:], xt[k][:, sti * 128:(sti + 1) * 128],
                        wvs[:, k * n * 128:(k + 1) * n * 128],
                        start=(k == 0), stop=(k == 7))
                sb = st.tile([128, n * 128], bf16, tag="kv_stage", bufs=3)
                nc.vector.tensor_copy(sb[:], ps[:])
                nc.sync.dma_start(vb[u][sti * 128:(sti + 1) * 128, :], sb[:])
            nc.gpsimd.collective_compute(
                "AllGather", mybir.AluOpType.bypass, replica_groups=RG,
                ins=[vb[u].opt()], outs=[vg[u].opt()])

        qt = [None] * PAIRS

        def q_proj(m, psP):
            ws = st.tile([128, 8 * 128], bf16, tag="wl", bufs=3)
            nc.sync.dma_start(ws.rearrange("p (k c) -> p k c", c=128),
                              wq4[:, m, :, :])
            ps = psP.tile([128, SQ], f32, tag="proj", bufs=2)
            for k in range(8):
                nc.tensor.matmul(ps[:], ws[:, k * 128:(k + 1) * 128],
                                 xt[k][:], start=(k == 0), stop=(k == 7))
            t = res.tile([128, SQ], bf16, tag=f"qt{m}", name=f"qt{m}")
            nc.vector.tensor_copy(t[:], ps[:])
            qt[m] = t

        with tc.tile_pool(name="psP", bufs=1, space="PSUM") as psP:
            # emit each unit's kT then V so the collectives fire in exactly
            # the order attention consumes them
            kt_proj(0, psP)
            kt_proj(1, psP)
            v_proj(0, psP)
            kt_proj(2, psP)
            kt_proj(3, psP)
            v_proj(1, psP)
            q_proj(0, psP)
            kt_proj(4, psP)
            kt_proj(5, psP)
            v_proj(2, psP)
            kt_proj(6, psP)
            kt_proj(7, psP)
            v_proj(3, psP)
            for m in range(1, PAIRS):
                q_proj(m, psP)

        # ---- attention (head pairs row-packed on the PE array) ----
        # pair slabs: rows 0..63 even head, 64..127 odd head
        attn = []
        for p in range(PAIRS):
            t = res.tile([128, SQ], bf16, tag=f"attn{p}", name=f"attn{p}")
            attn.append(t)

        # out-projection weights (pair-stacked rows: odd heads at
        # partitions 64..127) and SBUF accumulators for incremental y
        wo5 = woutT.rearrange("(pp r) (o c) -> r o pp c", r=128, c=512)
        wo = []
        for och in range(2):
            w = res.tile([128, PAIRS * 512], bf16, tag=f"wo{och}",
                         name=f"wo{och}")
            nc.sync.dma_start(
                w.rearrange("r (pp c) -> r pp c", c=512), wo5[:, och])
            wo.append(w)
        with tc.tile_pool(name="psA", bufs=1, space="PSUM") as psA:
            for p in range(PAIRS):
                u, i = unit_of[p]
                n = len(UNITS[u])
                # pair K^T strip [128, 4096]: rows 0..63 head 2p, 64..127
                # head 2p+1 — matches qt[p] halves and tile_position rows
                ktg3 = ktg[u].rearrange("(c i r) q -> i r c q",
                                        i=n, r=128)
                kth = st.tile([128, S], bf16, tag="kth", bufs=3)
                nc.gpsimd.dma_start(
                    kth.rearrange("r (c q) -> r c q", q=SQ), ktg3[i])
                vg3 = vg[u].rearrange("(t q) (i hh c) -> i hh q t c", q=128,
                                      i=n, c=HD)[i]
                vah = []
                for e in range(2):
                    # memset 1.0 first: the data DMA then fills cols 0..63 of
                    # each 65-wide block, leaving col 64 as the ones column
                    va = st.tile([128, KTILES * VAUG], bf16, tag="vah",
                                 bufs=6)
                    nc.vector.memset(va[:], 1.0)
                    nc.gpsimd.dma_start(
                        va.rearrange("q (t c) -> q t c", c=VAUG)[:, :, 0:HD],
                        vg3[e])
                    vah.append(va)

                pv = [psA.tile([128, 512], f32, tag="pv", bufs=2,
                               name=f"pv{p}_{e}") for e in range(2)]

                # slot stream: (t, even), (t, odd) pairs; exp groups of 3.
                # Emit the PV matmuls RA groups behind the score/exp stream:
                # the in-order PE queue then has independent scores work
                # ahead of the first PV, which may wait on the V gather.
                RA = 8
                slots = [(t, e) for t in range(KTILES) for e in range(2)]
                groups = [slots[gs:gs + 3]
                          for gs in range(0, len(slots), 3)]
                pts = []

                def emit_scores(group):
                    gw = 512 * len(group)
                    sc = psA.tile([128, 1536], f32, tag="sc", bufs=2)
                    for idx, (t, e) in enumerate(group):
                        nc.tensor.matmul(
                            sc[:, idx * 512:(idx + 1) * 512],
                            kth[e * 64:(e + 1) * 64, t * 128:(t + 1) * 128],
                            qt[p][e * 64:(e + 1) * 64, :],
                            start=True, stop=True,
                            tile_position=(e * 64, 0))
                    pt = st.tile([128, 1536], bf16, tag="pt", bufs=RA + 2)
                    nc.scalar.activation(pt[:, 0:gw], sc[:, 0:gw],
                                         mybir.ActivationFunctionType.Exp,
                                         scale=float(SCALE))
                    pts.append(pt)

                def emit_pv(group, pt):
                    for idx, (t, e) in enumerate(group):
                        nc.tensor.matmul(
                            pv[e][0:VAUG, :],
                            vah[e][:, t * VAUG:(t + 1) * VAUG],
                            pt[:, idx * 512:(idx + 1) * 512],
                            start=(t == 0), stop=(t == KTILES - 1))

                for gi, group in enumerate(groups):
                    emit_scores(group)
                    if gi >= RA:
                        emit_pv(groups[gi - RA], pts[gi - RA])
                for gi in range(len(groups) - RA, len(groups)):
                    emit_pv(groups[gi], pts[gi])

                # normalize: out_head = pv_data / l  (l = ones-column row 64).
                # Evacuate pv to SBUF right away so the PSUM slots free for
                # the next pair; the normalize chain then runs off-PSUM.
                # (partition_broadcast reads the tile's partition 0, so bounce
                # the l row down to partition 0 via DMA first)
                for e in range(2):
                    pvs = st.tile([VAUG, 512], f32, tag="pvs", bufs=4)
                    nc.vector.tensor_copy(pvs[:], pv[e][0:VAUG, :])
                    l0 = st.tile([1, 512], f32, tag="l0", bufs=2)
                    nc.sync.dma_start(l0[:], pvs[64:65, :])
                    lb = st.tile([64, 512], f32, tag="lb", bufs=2)
                    nc.gpsimd.partition_broadcast(lb[:], l0[:])
                    rb = st.tile([64, 512], f32, tag="rb", bufs=2)
                    nc.vector.reciprocal_approx_fast(rb[:], lb[:])
                    if e == 0:
                        nc.vector.tensor_mul(attn[p][0:64, :],
                                             pvs[0:64, :], rb[:])
                    else:
                        ao = st.tile([64, SQ], bf16, tag="ao", bufs=2)
                        nc.vector.tensor_mul(ao[:], pvs[0:64, :], rb[:])
                        nc.sync.dma_start(attn[p][64:128, :], ao[:])

        # ---- out projection: y[s, o] = sum_h attn_h^T.T @ woutT[h rows],
        # row-packed pairs -> two accumulators (even/odd rows), then add ----
        with tc.tile_pool(name="psY", bufs=1, space="PSUM") as psY:
            for sti in range(4):
                for och in range(2):
                    psa = psY.tile([128, 512], f32, tag="ya", bufs=4)
                    psb = psY.tile([128, 512], f32, tag="yb", bufs=4)
                    for p in range(PAIRS):
                        nc.tensor.matmul(
                            psa[:], attn[p][0:64, sti * 128:(sti + 1) * 128],
                            wo[och][0:64, p * 512:(p + 1) * 512],
                            start=(p == 0), stop=(p == PAIRS - 1),
                            tile_position=(0, 0))
                        nc.tensor.matmul(
                            psb[:], attn[p][64:128, sti * 128:(sti + 1) * 128],
                            wo[och][64:128, p * 512:(p + 1) * 512],
                            start=(p == 0), stop=(p == PAIRS - 1),
                            tile_position=(64, 0))
                    ya = st.tile([128, 512], f32, tag="ya_sb", bufs=3)
                    nc.vector.tensor_copy(ya[:], psa[:])
                    ysb = st.tile([128, 512], f32, tag="ysb", bufs=3)
                    nc.vector.tensor_add(ysb[:], psb[:], ya[:])
                    nc.sync.dma_start(
                        y[sti * 128:(sti + 1) * 128,
                          och * 512:(och + 1) * 512], ysb[:])


def _get_nc():
    if "nc" not in _cache:
        _cache["nc"] = _build()
    return _cache["nc"]


def kernel(x, W_qkv, W_out, _trace=False):
    from concourse.bass_utils import run_bass_kernel_spmd

    nc = _get_nc()
    bf16 = ml_dtypes.bfloat16

    x = np.asarray(x)
    xTf = np.ascontiguousarray(x.reshape(S, HID).T).astype(bf16)   # [HID, S]
    wqkvT = np.ascontiguousarray(np.asarray(W_qkv).T).astype(bf16)
    woutT = np.ascontiguousarray(np.asarray(W_out).T).astype(bf16)

    in_maps = []
    for c in range(NCORES):
        in_maps.append({
            "xT": np.ascontiguousarray(xTf[:, c * SQ:(c + 1) * SQ]),
            "wqkvT": wqkvT,
            "woutT": woutT,
        })
    res = run_bass_kernel_spmd(nc, in_maps, core_ids=list(range(NCORES)),
                               trace=_trace)
    out = np.concatenate([res.results[c]["y"] for c in range(NCORES)],
                         axis=0)
    out = out.reshape(1, S, HID).astype(np.float32)
    if _trace:
        kernel.last_results = res
    return out
